# revision 17
# baseline (speedup 1.0000x reference)
"""Trainium2 Bass kernel for ViT-style attention block with RoPE.

Problem: x(64,197,1024), qkv(3072x1024)+b, proj(1024x1024)+b, H=16 heads,
RoPE (interleaved pairs, tiled cos/sin tables) on all tokens but CLS.

Strategy: data-parallel over batch across 8 cores (8 items each, no
collectives). Host pre-transposes all operands so the device only runs
matmuls / softmax / RoPE in "transposed" layouts:

  - qk part:  qkT[f, t] = Wqk^T stationary x xT moving   (features on partitions)
  - v part:   v[t, f]   = xT stationary x Wv moving      (tokens on partitions)
  - scores:   scT[j, i] = kT(lhsT) x qT(rhs), two heads packed into PE rows
              0:64 / 64:128 (row-group concurrency)
  - softmax:  exp on ScalarE (scale=1/8, no max subtraction; |logits|<~5),
              denominators via a ones-column appended to v (row 64 of AV psum),
              normalization = reciprocal + K=1 broadcast matmul + DVE mult
  - RoPE:     q' = (q+b)*cos + (P(q+b))*sinS where P is a 128x128 block-swap
              permutation done on the TensorEngine; sign and d-permutation
              folded into host-built tables
  - v bias:   folded into proj bias on host (attn rows sum to 1)
  - proj:     yT = Wproj^T stationary x concatT moving, bias on ScalarE

Everything that is position/shape-only (rope tables, permutations) is
precomputed here with numpy; only the 5 runtime inputs flow in.
"""

import sys

for _p in ("/opt/trn_rl_repo", "/opt/pypackages"):
    if _p not in sys.path:
        sys.path.append(_p)

import numpy as np
import ml_dtypes

import concourse.bass as bass
import concourse.tile as tile
from concourse import bacc
from concourse import mybir

F32 = mybir.dt.float32
BF16 = mybir.dt.bfloat16
BF16_NP = ml_dtypes.bfloat16

# Problem constants (hardcoded per the contract)
B, N, C = 64, 197, 1024
H, D = 16, 64
E = 1  # CLS tokens
THETA = 10000.0
N_CORES = 8
NI = B // N_CORES  # items per core = 8
NT = NI * N  # tokens per core = 1576
S = N  # 197
W = 2 * S  # pair width = 394
NPAIR = NI // 2  # 4
P = 128


def _host_tables():
    """RoPE cos/sin in device layout + permutations, all position-only."""
    seq = (224 // 16) ** 2  # 196
    exp = np.arange(0, D, 2, dtype=np.float64) / -D
    base = THETA**exp  # (32,)
    t = np.arange(seq, dtype=np.float64)
    f0 = np.outer(t, base)  # (196, 32)
    f = np.concatenate([f0, f0], axis=-1)  # (196, 64) "tiled"
    cos_ref = np.cos(f)
    sin_ref = np.sin(f)

    # permutation: new dd<32 -> orig 2dd (x0), new dd>=32 -> orig 2(dd-32)+1 (x1)
    perm = np.empty(D, dtype=np.int64)
    perm[:32] = np.arange(32) * 2
    perm[32:] = np.arange(32) * 2 + 1

    # per-token columns for an item: col 0 = CLS (cos=1, sin=0), cols 1..196 = rope
    cos_item = np.ones((D, S), dtype=np.float64)
    sin_item = np.zeros((D, S), dtype=np.float64)
    cos_item[:, 1:] = cos_ref[:, perm].T
    sin_item[:, 1:] = sin_ref[:, perm].T
    # fold rotate-half signs into sin: rot[dd<32] = -q[dd+32], rot[dd>=32] = +q[dd-32]
    sinS_item = sin_item.copy()
    sinS_item[:32, :] *= -1.0

    # pair-width, replicated for the 2 heads in a 128-partition tile
    cosT = np.tile(cos_item, (2, 2)).astype(BF16_NP)  # [128, 394]
    sinST = np.tile(sinS_item, (2, 2)).astype(BF16_NP)  # [128, 394]

    # 128x128 swap permutation (block swap +-32 within each 64-head-half),
    # already transposed for use as lhsT: rot = P @ q  ->  lhsT = P.T
    Pm = np.zeros((P, P), dtype=np.float32)
    for p in range(P):
        src = 64 * (p // 64) + ((p % 64) + 32) % 64
        Pm[p, src] = 1.0
    pmatT = Pm.T.astype(BF16_NP)  # [K=128, M=128]

    return perm, cosT, sinST, pmatT


def _pack_weights(qkv_w, qkv_b, proj_w, proj_b, perm):
    """Host-side weight packing into device layouts (all numpy, one-time)."""
    # feature permutation for q/k heads: rows of qkv_w within each head
    qk_perm = np.concatenate(
        [h * D + perm for h in range(2 * H)]  # q heads then k heads
    )
    wqk = qkv_w[:2048][qk_perm]  # (2048, 1024) permuted
    bqk = qkv_b[:2048][qk_perm]  # (2048,)
    wv = qkv_w[2048:]  # (1024, 1024)
    bv = qkv_b[2048:]

    wqk_T = np.ascontiguousarray(wqk.T).astype(BF16_NP)  # [1024, 2048]
    wv_T = np.ascontiguousarray(wv.T).astype(BF16_NP)  # [1024, 1024]
    proj_wT = np.ascontiguousarray(proj_w.T).astype(BF16_NP)  # [1024, 1024]

    # biases in [128, ftile] per-partition layout
    bqk_dev = np.ascontiguousarray(bqk.reshape(16, 128).T).astype(np.float32)
    # v bias folded into proj bias: y = concat@W^T + (W@bv + pb)
    beff = proj_w.astype(np.float64) @ bv.astype(np.float64) + proj_b
    beff_dev = np.ascontiguousarray(beff.reshape(8, 128).T).astype(np.float32)
    return wqk_T, wv_T, proj_wT, bqk_dev, beff_dev


def build_nc(n_items=NI, phases=4, att_sub=4, att_ops=None):
    """Build the per-core Bass graph. SPMD: same graph on all cores.

    phases: 1=qk only, 2=+v, 3=+att, 4=full (debug bisection aid; truncated
    variants store intermediate tiles to `out` so work isn't dead-code).
    """
    nt = n_items * S
    npair = n_items // 2
    nc = bacc.Bacc(None, target_bir_lowering=False, debug=False)

    xT = nc.declare_dram_parameter("xT", [C, nt], BF16, isOutput=False)
    wqk = nc.declare_dram_parameter("wqk", [C, 2048], BF16, isOutput=False)
    wv = nc.declare_dram_parameter("wv", [C, C], BF16, isOutput=False)
    wpr = nc.declare_dram_parameter("wpr", [C, C], BF16, isOutput=False)
    pmat = nc.declare_dram_parameter("pmat", [P, P], BF16, isOutput=False)
    bqk = nc.declare_dram_parameter("bqk", [P, 16], F32, isOutput=False)
    beff = nc.declare_dram_parameter("beff", [P, 8], F32, isOutput=False)
    cosT = nc.declare_dram_parameter("cosT", [P, W], BF16, isOutput=False)
    sinST = nc.declare_dram_parameter("sinST", [P, W], BF16, isOutput=False)
    out = nc.declare_dram_parameter("out", [C, nt], F32, isOutput=True)

    Exp = mybir.ActivationFunctionType.Exp
    Ident = mybir.ActivationFunctionType.Identity

    with tile.TileContext(nc) as tc:
        with (
            tc.tile_pool(name="const", bufs=1) as const,
            tc.tile_pool(name="xp", bufs=2) as xp,
            tc.tile_pool(name="roped", bufs=2) as rp,
            tc.tile_pool(name="vp", bufs=2) as vp,
            tc.tile_pool(name="work", bufs=4) as wk,
            tc.tile_pool(name="ep", bufs=4) as ep,
            tc.tile_pool(name="cc", bufs=2) as cc,
            tc.tile_pool(name="yp", bufs=3) as yp,
            tc.tile_pool(name="psA", bufs=2, space="PSUM") as psA,
            tc.tile_pool(name="psS", bufs=2, space="PSUM") as psS,
            tc.tile_pool(name="psV", bufs=1, space="PSUM") as psV,
        ):
            # ---- one-time loads ----
            # Order matters: pair-0 x first, qk weights split per k-tile so
            # the first chains start while later tiles stream in; v/proj
            # weights are not needed until tens of us later.
            xT3 = xT.rearrange("(o p) t -> p o t", p=P)
            out3 = out.rearrange("(o p) t -> p o t", p=P)
            x_first = xp.tile([P, 8, W], BF16, tag="x")
            for kt in range(8):
                nc.sync.dma_start(x_first[:, kt], xT3[:, kt, 0:W])

            wqk_sb = const.tile([P, 8, 2048], BF16)
            wqk3 = wqk.rearrange("(o p) f -> p o f", p=P)
            for kt in range(8):
                nc.sync.dma_start(wqk_sb[:, kt], wqk3[:, kt])
            pmat_sb = const.tile([P, P], BF16)
            nc.sync.dma_start(pmat_sb, pmat[:, :])
            bqk_sb = const.tile([P, 16], F32)
            nc.sync.dma_start(bqk_sb, bqk[:, :])
            beff_sb = const.tile([P, 8], F32)
            nc.sync.dma_start(beff_sb, beff[:, :])
            cos_sb = const.tile([P, W], BF16)
            nc.sync.dma_start(cos_sb, cosT[:, :])
            sin_sb = const.tile([P, W], BF16)
            nc.sync.dma_start(sin_sb, sinST[:, :])
            wv_sb = const.tile([P, 8, C], BF16)
            nc.sync.dma_start(wv_sb, wv.rearrange("(o p) f -> p o f", p=P))
            wpr_sb = const.tile([P, 8, C], BF16)
            nc.sync.dma_start(wpr_sb, wpr.rearrange("(o p) f -> p o f", p=P))

            for pr in range(npair):
                tok = pr * W  # global token col of this pair

                if pr == 0:
                    x_sb = x_first
                else:
                    x_sb = xp.tile([P, 8, W], BF16, tag="x")
                    for kt in range(8):
                        nc.sync.dma_start(
                            x_sb[:, kt], xT3[:, kt, tok : tok + W]
                        )

                # ---- phase QK: qkT = Wqk^T x, bias, rope ----
                roped = rp.tile([P, 16, W], BF16)
                for ft in range(16):
                    ps_qk = psA.tile([P, 512], F32, tag="mm")
                    for kt in range(8):
                        nc.tensor.matmul(
                            ps_qk[:, :W],
                            wqk_sb[:, kt, ft * P : (ft + 1) * P],
                            x_sb[:, kt, :],
                            start=(kt == 0),
                            stop=(kt == 7),
                        )
                    tmp = wk.tile([P, W], BF16, tag="tmp")
                    nc.scalar.activation(
                        tmp, ps_qk[:, :W], Ident, bias=bqk_sb[:, ft : ft + 1]
                    )
                    ps_rot = psA.tile([P, 512], F32, tag="mm")
                    nc.tensor.matmul(
                        ps_rot[:, :W], pmat_sb, tmp, start=True, stop=True
                    )
                    acc = wk.tile([P, W], BF16, tag="acc")
                    nc.vector.tensor_mul(acc, tmp, cos_sb)
                    rot2 = wk.tile([P, W], BF16, tag="rot2")
                    nc.vector.tensor_mul(rot2, ps_rot[:, :W], sin_sb)
                    nc.vector.tensor_add(roped[:, ft, :], acc, rot2)

                if phases == 1:
                    for ft in range(8):
                        dbg = yp.tile([P, W], F32)
                        nc.vector.tensor_copy(dbg, roped[:, ft, :])
                        nc.sync.dma_start(out3[:, ft, tok : tok + W], dbg)
                    continue

                # ---- phase V: v[t, f] = x^T Wv, + ones column ----
                v65 = []  # [item][jtile] -> [P, 16, 65] bf16
                for it2 in range(2):
                    tiles = []
                    for tt in range(2):
                        pcount = 128 if tt == 0 else 69
                        vt = vp.tile([P, 16, 128], BF16, tag=f"v{it2}{tt}")
                        for nk in range(2):
                            ps_v = psA.tile([P, 512], F32, tag="mm")
                            for kt in range(8):
                                nc.tensor.matmul(
                                    ps_v[:pcount, :],
                                    x_sb[
                                        :,
                                        kt,
                                        it2 * S + tt * P : it2 * S + tt * P + pcount,
                                    ],
                                    wv_sb[:, kt, nk * 512 : (nk + 1) * 512],
                                    start=(kt == 0),
                                    stop=(kt == 7),
                                )
                            nc.vector.tensor_copy(
                                vt[:pcount, nk * 8 : (nk + 1) * 8, 0:64],
                                ps_v[:pcount, :].rearrange(
                                    "p (h d) -> p h d", d=64
                                ),
                            )
                        nc.any.memset(vt[:pcount, :, 64:128], 1.0)
                        tiles.append(vt)
                    v65.append(tiles)

                if phases == 2:
                    for it2 in range(2):
                        for tt in range(2):
                            dbg = yp.tile([P, W], F32)
                            nc.vector.tensor_copy(
                                dbg[:, 0:390], v65[it2][tt][:, 0:6, :].rearrange("p a b -> p (a b)")
                            )
                            nc.sync.dma_start(
                                out3[:, it2 * 2 + tt, tok : tok + W], dbg
                            )
                    continue

                # ---- phase ATT ----
                # PSUM rule (hw): each bank gets exactly ONE accumulation
                # chain writing ONE region. Head pairs pack across the two
                # banks of a [128,1024] tile; reads use strided 3D APs.
                concat = cc.tile([P, 8, W], BF16)
                for it2 in range(2):
                    ts = it2 * S  # token col within pair
                    for hp in range(8):
                        hA, hB = 2 * hp, 2 * hp + 1
                        sc0 = psS.tile([P, 1024], F32, tag="sc")  # jt0, 2 banks
                        sc1 = psS.tile([P, 1024], F32, tag="sc")  # jt1, 2 banks
                        for h, bk in ((hA, 0), (hB, 1)):
                            hb = 64 * (h % 2)
                            kT = roped[hb : hb + 64, 8 + h // 2, ts : ts + S]
                            qT = roped[hb : hb + 64, h // 2, ts : ts + S]
                            nc.tensor.matmul(
                                sc0[:, bk * 512 : bk * 512 + S],
                                kT[:, 0:P],
                                qT,
                                start=True,
                                stop=True,
                            )
                            nc.tensor.matmul(
                                sc1[0:69, bk * 512 : bk * 512 + S],
                                kT[:, P:S],
                                qT,
                                start=True,
                                stop=True,
                            )
                        e0 = ep.tile([P, 2, S], BF16, tag="e0")
                        e1 = ep.tile([P, 2, S], BF16, tag="e1")
                        sc0v = sc0.rearrange("p (b c) -> p b c", b=2)[:, :, 0:S]
                        sc1v = sc1.rearrange("p (b c) -> p b c", b=2)[:, :, 0:S]
                        nc.scalar.activation(e0, sc0v, Exp, scale=0.125)
                        nc.scalar.activation(
                            e1[0:69], sc1v[0:69], Exp, scale=0.125
                        )
                        av = psV.tile([P, 1024], F32, tag="av")  # 2 banks
                        for h, bk in ((hA, 0), (hB, 1)):
                            nc.tensor.matmul(
                                av[:, bk * 512 : bk * 512 + S],
                                v65[it2][0][:, h, :],
                                e0[:, bk, :],
                                start=True,
                                stop=False,
                            )
                            nc.tensor.matmul(
                                av[:, bk * 512 : bk * 512 + S],
                                v65[it2][1][0:69, h, :],
                                e1[0:69, bk, :],
                                start=False,
                                stop=True,
                            )
                        avv = av.rearrange("p (b c) -> p b c", b=2)
                        # normalization: rows 64:128 = sum replicated (ones
                        # columns in v65); fast reciprocal + per-head mult
                        ssum = wk.tile([64, 2, S], F32, tag="ssum")
                        nc.scalar.copy(ssum, avv[64:128, :, 0:S])
                        rb = wk.tile([64, 2, S], F32, tag="rb")
                        nc.vector.reciprocal_approx_fast(rb, ssum)
                        for h, bk in ((hA, 0), (hB, 1)):
                            nc.vector.tensor_mul(
                                concat[
                                    64 * (h % 2) : 64 * (h % 2) + 64, hp, ts : ts + S
                                ],
                                avv[0:64, bk, 0:S],
                                rb[:, bk, :],
                            )

                if phases == 3:
                    for ft in range(8):
                        dbg = yp.tile([P, W], F32)
                        nc.vector.tensor_copy(dbg, concat[:, ft, :])
                        nc.sync.dma_start(out3[:, ft, tok : tok + W], dbg)
                    continue

                # ---- phase PROJ ----
                for ft in range(8):
                    ps_y = psS.tile([P, 1024], F32, tag="sc")
                    for kt in range(8):
                        nc.tensor.matmul(
                            ps_y[:, :W],
                            wpr_sb[:, kt, ft * P : (ft + 1) * P],
                            concat[:, kt, :],
                            start=(kt == 0),
                            stop=(kt == 7),
                        )
                    y_sb = yp.tile([P, W], F32)
                    nc.scalar.activation(
                        y_sb, ps_y[:, :W], Ident, bias=beff_sb[:, ft : ft + 1]
                    )
                    nc.sync.dma_start(out3[:, ft, tok : tok + W], y_sb)

    nc.compile()
    return nc


def host_pack_inputs(x, qkv_w, qkv_b, proj_w, proj_b, n_items=NI):
    """Build per-core in_maps (host-side layout only, no math on x)."""
    perm, cosT, sinST, pmatT = _host_tables()
    wqk_T, wv_T, proj_wT, bqk_dev, beff_dev = _pack_weights(
        qkv_w, qkv_b, proj_w, proj_b, perm
    )
    shared = {
        "wqk": wqk_T,
        "wv": wv_T,
        "wpr": proj_wT,
        "pmat": np.ascontiguousarray(pmatT),
        "bqk": bqk_dev,
        "beff": beff_dev,
        "cosT": np.ascontiguousarray(cosT),
        "sinST": np.ascontiguousarray(sinST),
    }
    n_cores = x.shape[0] // n_items
    in_maps = []
    for c in range(n_cores):
        xs = x[c * n_items : (c + 1) * n_items]  # [ni, 197, 1024]
        xTs = np.ascontiguousarray(
            xs.reshape(n_items * S, C).T.astype(BF16_NP)
        )  # [1024, nt]
        in_maps.append({"xT": xTs, **shared})
    return in_maps


def unpack_output(results, n_items=NI):
    """results: list of per-core {'out': [1024, nt]} -> full (B, N, C) f32."""
    outs = []
    for r in results:
        yT = r["out"]  # [1024, nt]
        outs.append(yT.T.reshape(n_items, S, C))
    return np.concatenate(outs, axis=0)


_CACHED = {}


def kernel(x, qkv_w, qkv_b, proj_w, proj_b):
    from concourse.bass_utils import run_bass_kernel_spmd

    x = np.asarray(x, dtype=np.float32)
    qkv_w = np.asarray(qkv_w, dtype=np.float32)
    qkv_b = np.asarray(qkv_b, dtype=np.float32)
    proj_w = np.asarray(proj_w, dtype=np.float32)
    proj_b = np.asarray(proj_b, dtype=np.float32)

    if "nc" not in _CACHED:
        _CACHED["nc"] = build_nc(NI)
    nc = _CACHED["nc"]
    in_maps = host_pack_inputs(x, qkv_w, qkv_b, proj_w, proj_b, NI)
    res = run_bass_kernel_spmd(nc, in_maps, core_ids=list(range(N_CORES)))
    return unpack_output(res.results, NI).astype(np.float32)


if __name__ == "__main__":
    pass


# revision 18
# speedup vs baseline: 1.0064x; 1.0064x over previous
"""Trainium2 Bass kernel for ViT-style attention block with RoPE.

Problem: x(64,197,1024), qkv(3072x1024)+b, proj(1024x1024)+b, H=16 heads,
RoPE (interleaved pairs, tiled cos/sin tables) on all tokens but CLS.

Strategy: data-parallel over batch across 8 cores (8 items each, no
collectives). Host pre-transposes all operands so the device only runs
matmuls / softmax / RoPE in "transposed" layouts:

  - qk part:  qkT[f, t] = Wqk^T stationary x xT moving   (features on partitions)
  - v part:   v[t, f]   = xT stationary x Wv moving      (tokens on partitions)
  - scores:   scT[j, i] = kT(lhsT) x qT(rhs), two heads packed into PE rows
              0:64 / 64:128 (row-group concurrency)
  - softmax:  exp on ScalarE (scale=1/8, no max subtraction; |logits|<~5),
              denominators via a ones-column appended to v (row 64 of AV psum),
              normalization = reciprocal + K=1 broadcast matmul + DVE mult
  - RoPE:     q' = (q+b)*cos + (P(q+b))*sinS where P is a 128x128 block-swap
              permutation done on the TensorEngine; sign and d-permutation
              folded into host-built tables
  - v bias:   folded into proj bias on host (attn rows sum to 1)
  - proj:     yT = Wproj^T stationary x concatT moving, bias on ScalarE

Everything that is position/shape-only (rope tables, permutations) is
precomputed here with numpy; only the 5 runtime inputs flow in.
"""

import sys

for _p in ("/opt/trn_rl_repo", "/opt/pypackages"):
    if _p not in sys.path:
        sys.path.append(_p)

import numpy as np
import ml_dtypes

import concourse.bass as bass
import concourse.tile as tile
from concourse import bacc
from concourse import mybir

F32 = mybir.dt.float32
BF16 = mybir.dt.bfloat16
BF16_NP = ml_dtypes.bfloat16

# Problem constants (hardcoded per the contract)
B, N, C = 64, 197, 1024
H, D = 16, 64
E = 1  # CLS tokens
THETA = 10000.0
N_CORES = 8
NI = B // N_CORES  # items per core = 8
NT = NI * N  # tokens per core = 1576
S = N  # 197
W = 2 * S  # pair width = 394
NPAIR = NI // 2  # 4
P = 128


def _host_tables():
    """RoPE cos/sin in device layout + permutations, all position-only."""
    seq = (224 // 16) ** 2  # 196
    exp = np.arange(0, D, 2, dtype=np.float64) / -D
    base = THETA**exp  # (32,)
    t = np.arange(seq, dtype=np.float64)
    f0 = np.outer(t, base)  # (196, 32)
    f = np.concatenate([f0, f0], axis=-1)  # (196, 64) "tiled"
    cos_ref = np.cos(f)
    sin_ref = np.sin(f)

    # permutation: new dd<32 -> orig 2dd (x0), new dd>=32 -> orig 2(dd-32)+1 (x1)
    perm = np.empty(D, dtype=np.int64)
    perm[:32] = np.arange(32) * 2
    perm[32:] = np.arange(32) * 2 + 1

    # per-token columns for an item: col 0 = CLS (cos=1, sin=0), cols 1..196 = rope
    cos_item = np.ones((D, S), dtype=np.float64)
    sin_item = np.zeros((D, S), dtype=np.float64)
    cos_item[:, 1:] = cos_ref[:, perm].T
    sin_item[:, 1:] = sin_ref[:, perm].T
    # fold rotate-half signs into sin: rot[dd<32] = -q[dd+32], rot[dd>=32] = +q[dd-32]
    sinS_item = sin_item.copy()
    sinS_item[:32, :] *= -1.0

    # pair-width, replicated for the 2 heads in a 128-partition tile
    cosT = np.tile(cos_item, (2, 2)).astype(BF16_NP)  # [128, 394]
    sinST = np.tile(sinS_item, (2, 2)).astype(BF16_NP)  # [128, 394]

    # 128x128 swap permutation (block swap +-32 within each 64-head-half),
    # already transposed for use as lhsT: rot = P @ q  ->  lhsT = P.T
    Pm = np.zeros((P, P), dtype=np.float32)
    for p in range(P):
        src = 64 * (p // 64) + ((p % 64) + 32) % 64
        Pm[p, src] = 1.0
    pmatT = Pm.T.astype(BF16_NP)  # [K=128, M=128]

    return perm, cosT, sinST, pmatT


def _pack_weights(qkv_w, qkv_b, proj_w, proj_b, perm):
    """Host-side weight packing into device layouts (all numpy, one-time)."""
    # feature permutation for q/k heads: rows of qkv_w within each head
    qk_perm = np.concatenate(
        [h * D + perm for h in range(2 * H)]  # q heads then k heads
    )
    wqk = qkv_w[:2048][qk_perm]  # (2048, 1024) permuted
    bqk = qkv_b[:2048][qk_perm]  # (2048,)
    wv = qkv_w[2048:]  # (1024, 1024)
    bv = qkv_b[2048:]

    wqk_T = np.ascontiguousarray(wqk.T).astype(BF16_NP)  # [1024, 2048]
    wv_T = np.ascontiguousarray(wv.T).astype(BF16_NP)  # [1024, 1024]
    proj_wT = np.ascontiguousarray(proj_w.T).astype(BF16_NP)  # [1024, 1024]

    # biases in [128, ftile] per-partition layout
    bqk_dev = np.ascontiguousarray(bqk.reshape(16, 128).T).astype(np.float32)
    # v bias folded into proj bias: y = concat@W^T + (W@bv + pb)
    beff = proj_w.astype(np.float64) @ bv.astype(np.float64) + proj_b
    beff_dev = np.ascontiguousarray(beff.reshape(8, 128).T).astype(np.float32)
    return wqk_T, wv_T, proj_wT, bqk_dev, beff_dev


def build_nc(n_items=NI, phases=4, att_sub=4, att_ops=None):
    """Build the per-core Bass graph. SPMD: same graph on all cores.

    phases: 1=qk only, 2=+v, 3=+att, 4=full (debug bisection aid; truncated
    variants store intermediate tiles to `out` so work isn't dead-code).
    """
    nt = n_items * S
    npair = n_items // 2
    nc = bacc.Bacc(None, target_bir_lowering=False, debug=False)

    xT = nc.declare_dram_parameter("xT", [C, nt], BF16, isOutput=False)
    wqk = nc.declare_dram_parameter("wqk", [C, 2048], BF16, isOutput=False)
    wv = nc.declare_dram_parameter("wv", [C, C], BF16, isOutput=False)
    wpr = nc.declare_dram_parameter("wpr", [C, C], BF16, isOutput=False)
    pmat = nc.declare_dram_parameter("pmat", [P, P], BF16, isOutput=False)
    bqk = nc.declare_dram_parameter("bqk", [P, 16], F32, isOutput=False)
    beff = nc.declare_dram_parameter("beff", [P, 8], F32, isOutput=False)
    cosT = nc.declare_dram_parameter("cosT", [P, W], BF16, isOutput=False)
    sinST = nc.declare_dram_parameter("sinST", [P, W], BF16, isOutput=False)
    out = nc.declare_dram_parameter("out", [C, nt], F32, isOutput=True)

    Exp = mybir.ActivationFunctionType.Exp
    Ident = mybir.ActivationFunctionType.Identity

    with tile.TileContext(nc) as tc:
        with (
            tc.tile_pool(name="const", bufs=1) as const,
            tc.tile_pool(name="xp", bufs=2) as xp,
            tc.tile_pool(name="roped", bufs=2) as rp,
            tc.tile_pool(name="vp", bufs=2) as vp,
            tc.tile_pool(name="work", bufs=4) as wk,
            tc.tile_pool(name="ep", bufs=4) as ep,
            tc.tile_pool(name="cc", bufs=2) as cc,
            tc.tile_pool(name="yp", bufs=3) as yp,
            tc.tile_pool(name="psA", bufs=2, space="PSUM") as psA,
            tc.tile_pool(name="psS", bufs=2, space="PSUM") as psS,
            tc.tile_pool(name="psV", bufs=1, space="PSUM") as psV,
        ):
            # ---- one-time loads ----
            # Order matters: pair-0 x first, qk weights split per k-tile so
            # the first chains start while later tiles stream in; v/proj
            # weights are not needed until tens of us later.
            xT3 = xT.rearrange("(o p) t -> p o t", p=P)
            out3 = out.rearrange("(o p) t -> p o t", p=P)
            x_first = xp.tile([P, 8, W], BF16, tag="x")
            for kt in range(8):
                nc.sync.dma_start(x_first[:, kt], xT3[:, kt, 0:W])

            wqk_sb = const.tile([P, 8, 2048], BF16)
            wqk3 = wqk.rearrange("(o p) f -> p o f", p=P)
            for kt in range(8):
                nc.sync.dma_start(wqk_sb[:, kt], wqk3[:, kt])
            pmat_sb = const.tile([P, P], BF16)
            nc.sync.dma_start(pmat_sb, pmat[:, :])
            bqk_sb = const.tile([P, 16], F32)
            nc.sync.dma_start(bqk_sb, bqk[:, :])
            beff_sb = const.tile([P, 8], F32)
            nc.sync.dma_start(beff_sb, beff[:, :])
            cos_sb = const.tile([P, W], BF16)
            nc.sync.dma_start(cos_sb, cosT[:, :])
            sin_sb = const.tile([P, W], BF16)
            nc.sync.dma_start(sin_sb, sinST[:, :])
            wv_sb = const.tile([P, 8, C], BF16)
            nc.sync.dma_start(wv_sb, wv.rearrange("(o p) f -> p o f", p=P))
            wpr_sb = const.tile([P, 8, C], BF16)
            nc.sync.dma_start(wpr_sb, wpr.rearrange("(o p) f -> p o f", p=P))

            for pr in range(npair):
                tok = pr * W  # global token col of this pair

                if pr == 0:
                    x_sb = x_first
                else:
                    x_sb = xp.tile([P, 8, W], BF16, tag="x")
                    nc.sync.dma_start(x_sb, xT3[:, :, tok : tok + W])

                # ---- phase QK: qkT = Wqk^T x, bias, rope ----
                roped = rp.tile([P, 16, W], BF16)
                for ft in range(16):
                    ps_qk = psA.tile([P, 512], F32, tag="mm")
                    for kt in range(8):
                        nc.tensor.matmul(
                            ps_qk[:, :W],
                            wqk_sb[:, kt, ft * P : (ft + 1) * P],
                            x_sb[:, kt, :],
                            start=(kt == 0),
                            stop=(kt == 7),
                        )
                    tmp = wk.tile([P, W], BF16, tag="tmp")
                    nc.scalar.activation(
                        tmp, ps_qk[:, :W], Ident, bias=bqk_sb[:, ft : ft + 1]
                    )
                    ps_rot = psA.tile([P, 512], F32, tag="mm")
                    nc.tensor.matmul(
                        ps_rot[:, :W], pmat_sb, tmp, start=True, stop=True
                    )
                    acc = wk.tile([P, W], BF16, tag="acc")
                    nc.vector.tensor_mul(acc, tmp, cos_sb)
                    rot2 = wk.tile([P, W], BF16, tag="rot2")
                    nc.vector.tensor_mul(rot2, ps_rot[:, :W], sin_sb)
                    nc.vector.tensor_add(roped[:, ft, :], acc, rot2)

                if phases == 1:
                    for ft in range(8):
                        dbg = yp.tile([P, W], F32)
                        nc.vector.tensor_copy(dbg, roped[:, ft, :])
                        nc.sync.dma_start(out3[:, ft, tok : tok + W], dbg)
                    continue

                # ---- phase V: v[t, f] = x^T Wv, + ones column ----
                v65 = []  # [item][jtile] -> [P, 16, 65] bf16
                for it2 in range(2):
                    tiles = []
                    for tt in range(2):
                        pcount = 128 if tt == 0 else 69
                        vt = vp.tile([P, 16, 128], BF16, tag=f"v{it2}{tt}")
                        for nk in range(2):
                            ps_v = psA.tile([P, 512], F32, tag="mm")
                            for kt in range(8):
                                nc.tensor.matmul(
                                    ps_v[:pcount, :],
                                    x_sb[
                                        :,
                                        kt,
                                        it2 * S + tt * P : it2 * S + tt * P + pcount,
                                    ],
                                    wv_sb[:, kt, nk * 512 : (nk + 1) * 512],
                                    start=(kt == 0),
                                    stop=(kt == 7),
                                )
                            nc.vector.tensor_copy(
                                vt[:pcount, nk * 8 : (nk + 1) * 8, 0:64],
                                ps_v[:pcount, :].rearrange(
                                    "p (h d) -> p h d", d=64
                                ),
                            )
                        nc.any.memset(vt[:pcount, :, 64:128], 1.0)
                        tiles.append(vt)
                    v65.append(tiles)

                if phases == 2:
                    for it2 in range(2):
                        for tt in range(2):
                            dbg = yp.tile([P, W], F32)
                            nc.vector.tensor_copy(
                                dbg[:, 0:390], v65[it2][tt][:, 0:6, :].rearrange("p a b -> p (a b)")
                            )
                            nc.sync.dma_start(
                                out3[:, it2 * 2 + tt, tok : tok + W], dbg
                            )
                    continue

                # ---- phase ATT ----
                # PSUM rule (hw): each bank gets exactly ONE accumulation
                # chain writing ONE region. Head pairs pack across the two
                # banks of a [128,1024] tile; reads use strided 3D APs.
                concat = cc.tile([P, 8, W], BF16)
                for it2 in range(2):
                    ts = it2 * S  # token col within pair
                    for hp in range(8):
                        hA, hB = 2 * hp, 2 * hp + 1
                        sc0 = psS.tile([P, 1024], F32, tag="sc")  # jt0, 2 banks
                        sc1 = psS.tile([P, 1024], F32, tag="sc")  # jt1, 2 banks
                        for h, bk in ((hA, 0), (hB, 1)):
                            hb = 64 * (h % 2)
                            kT = roped[hb : hb + 64, 8 + h // 2, ts : ts + S]
                            qT = roped[hb : hb + 64, h // 2, ts : ts + S]
                            nc.tensor.matmul(
                                sc0[:, bk * 512 : bk * 512 + S],
                                kT[:, 0:P],
                                qT,
                                start=True,
                                stop=True,
                            )
                            nc.tensor.matmul(
                                sc1[0:69, bk * 512 : bk * 512 + S],
                                kT[:, P:S],
                                qT,
                                start=True,
                                stop=True,
                            )
                        e0 = ep.tile([P, 2, S], BF16, tag="e0")
                        e1 = ep.tile([P, 2, S], BF16, tag="e1")
                        sc0v = sc0.rearrange("p (b c) -> p b c", b=2)[:, :, 0:S]
                        sc1v = sc1.rearrange("p (b c) -> p b c", b=2)[:, :, 0:S]
                        nc.scalar.activation(e0, sc0v, Exp, scale=0.125)
                        nc.scalar.activation(
                            e1[0:69], sc1v[0:69], Exp, scale=0.125
                        )
                        av = psV.tile([P, 1024], F32, tag="av")  # 2 banks
                        for h, bk in ((hA, 0), (hB, 1)):
                            nc.tensor.matmul(
                                av[:, bk * 512 : bk * 512 + S],
                                v65[it2][0][:, h, :],
                                e0[:, bk, :],
                                start=True,
                                stop=False,
                            )
                            nc.tensor.matmul(
                                av[:, bk * 512 : bk * 512 + S],
                                v65[it2][1][0:69, h, :],
                                e1[0:69, bk, :],
                                start=False,
                                stop=True,
                            )
                        avv = av.rearrange("p (b c) -> p b c", b=2)
                        # normalization: rows 64:128 = sum replicated (ones
                        # columns in v65); fast reciprocal + per-head mult
                        ssum = wk.tile([64, 2, S], F32, tag="ssum")
                        nc.scalar.copy(ssum, avv[64:128, :, 0:S])
                        rb = wk.tile([64, 2, S], F32, tag="rb")
                        nc.vector.reciprocal_approx_fast(rb, ssum)
                        for h, bk in ((hA, 0), (hB, 1)):
                            nc.vector.tensor_mul(
                                concat[
                                    64 * (h % 2) : 64 * (h % 2) + 64, hp, ts : ts + S
                                ],
                                avv[0:64, bk, 0:S],
                                rb[:, bk, :],
                            )

                if phases == 3:
                    for ft in range(8):
                        dbg = yp.tile([P, W], F32)
                        nc.vector.tensor_copy(dbg, concat[:, ft, :])
                        nc.sync.dma_start(out3[:, ft, tok : tok + W], dbg)
                    continue

                # ---- phase PROJ (per item for finer overlap) ----
                for it2 in range(2):
                    ts = it2 * S
                    for ft in range(8):
                        ps_y = psS.tile([P, 1024], F32, tag="sc")
                        for kt in range(8):
                            nc.tensor.matmul(
                                ps_y[:, 0:S],
                                wpr_sb[:, kt, ft * P : (ft + 1) * P],
                                concat[:, kt, ts : ts + S],
                                start=(kt == 0),
                                stop=(kt == 7),
                            )
                        y_sb = yp.tile([P, S], F32)
                        nc.scalar.activation(
                            y_sb, ps_y[:, 0:S], Ident, bias=beff_sb[:, ft : ft + 1]
                        )
                        nc.sync.dma_start(
                            out3[:, ft, tok + ts : tok + ts + S], y_sb
                        )

    nc.compile()
    return nc


def host_pack_inputs(x, qkv_w, qkv_b, proj_w, proj_b, n_items=NI):
    """Build per-core in_maps (host-side layout only, no math on x)."""
    perm, cosT, sinST, pmatT = _host_tables()
    wqk_T, wv_T, proj_wT, bqk_dev, beff_dev = _pack_weights(
        qkv_w, qkv_b, proj_w, proj_b, perm
    )
    shared = {
        "wqk": wqk_T,
        "wv": wv_T,
        "wpr": proj_wT,
        "pmat": np.ascontiguousarray(pmatT),
        "bqk": bqk_dev,
        "beff": beff_dev,
        "cosT": np.ascontiguousarray(cosT),
        "sinST": np.ascontiguousarray(sinST),
    }
    n_cores = x.shape[0] // n_items
    in_maps = []
    for c in range(n_cores):
        xs = x[c * n_items : (c + 1) * n_items]  # [ni, 197, 1024]
        xTs = np.ascontiguousarray(
            xs.reshape(n_items * S, C).T.astype(BF16_NP)
        )  # [1024, nt]
        in_maps.append({"xT": xTs, **shared})
    return in_maps


def unpack_output(results, n_items=NI):
    """results: list of per-core {'out': [1024, nt]} -> full (B, N, C) f32."""
    outs = []
    for r in results:
        yT = r["out"]  # [1024, nt]
        outs.append(yT.T.reshape(n_items, S, C))
    return np.concatenate(outs, axis=0)


_CACHED = {}


def kernel(x, qkv_w, qkv_b, proj_w, proj_b):
    from concourse.bass_utils import run_bass_kernel_spmd

    x = np.asarray(x, dtype=np.float32)
    qkv_w = np.asarray(qkv_w, dtype=np.float32)
    qkv_b = np.asarray(qkv_b, dtype=np.float32)
    proj_w = np.asarray(proj_w, dtype=np.float32)
    proj_b = np.asarray(proj_b, dtype=np.float32)

    if "nc" not in _CACHED:
        _CACHED["nc"] = build_nc(NI)
    nc = _CACHED["nc"]
    in_maps = host_pack_inputs(x, qkv_w, qkv_b, proj_w, proj_b, NI)
    res = run_bass_kernel_spmd(nc, in_maps, core_ids=list(range(N_CORES)))
    return unpack_output(res.results, NI).astype(np.float32)


if __name__ == "__main__":
    pass
